# revision 23
# baseline (speedup 1.0000x reference)
"""AdaptiveStdPool2d kernel for Trainium2 (8 NeuronCores, data-parallel).

Input  x: [32, 64, 512, 80] f32
Output:   [32, 64, 8, 10] f32  (mean/std interleaved along height)

Math: per (b, c), split H=512 into 4 windows of 128 and W=80 into 10
windows of 8; out[b,c,2*oh,ow] = mean of 128x8 window, out[b,c,2*oh+1,ow]
= sqrt(biased_var + 1e-14).

Strategy (per core, shard = 4 batches -> 256 (b,c) rows):
- partition dim = (b,c) row (2 tiles of 128), free dim = flattened (h,w).
- per (bc-tile, oh): one big contiguous DMA slab [128, 10240] (40 KB/row).
- stats: per window, 2x BNStats (512-elem halves; HW flattens the
  multi-dim AP into one group) + BNStatsAggregate writing (mean, var)
  directly into the interleaved output layout. Single DVE pass gives
  both stats; ACT only does the final sqrt(var + eps) in place.
- last slab splits windows across DVE (bn_stats) and ACT (Square/Copy +
  accum_out) so the drain tail is shared by both engines.
"""

import os
import numpy as np

B, C, H, W = 32, 64, 512, 80
OUT_H2, OUT_W = 4, 10
WH, WW = H // OUT_H2, W // OUT_W  # 128, 8
EPS = 1e-14
NWIN = WH * WW                   # 1024 elements per window

N_CORES = 8
B_SH = B // N_CORES          # 4 batches per core
BC = B_SH * C                # 256 rows per core
HW = H * W                   # 40960
SLAB = WH * W                # 10240 elements per (oh) slab
OUT_FREE = 2 * OUT_H2 * OUT_W  # 80 output elements per (b,c)

_CACHE = {}
LAST_RESULTS = None


def _build():
    import concourse.bacc as bacc
    import concourse.tile as tile
    from concourse import mybir

    nc = bacc.Bacc("TRN2", target_bir_lowering=False, debug=False)

    x = nc.dram_tensor("x", [BC, HW], mybir.dt.float32, kind="ExternalInput")
    out = nc.dram_tensor("out", [BC, OUT_FREE], mybir.dt.float32,
                         kind="ExternalOutput")

    P = 128
    NT = BC // P  # 2 bc-tiles
    INV_N = 1.0 / NWIN

    def bn_stats_raw(in_ap, out_ap):
        # Raw InstBNStats: HW computes one 6-stat group over the whole
        # (multi-dim) input AP; bass's wrapper would reject this shape.
        nc.vector.add_instruction(
            mybir.InstBNStats(
                name=nc.get_next_instruction_name(),
                ins=[nc.vector.lower_ap(in_ap)],
                outs=[nc.vector.lower_ap(out_ap)],
            )
        )

    with tile.TileContext(nc) as tc:
        with (
            tc.tile_pool(name="slabs", bufs=4) as slabs,
            tc.tile_pool(name="scratch", bufs=2) as scratch_pool,
            tc.tile_pool(name="small", bufs=4) as small,
            tc.tile_pool(name="res", bufs=2) as res_pool,
            tc.tile_pool(name="singles", bufs=1) as singles,
        ):
            eps_t = singles.tile([P, 1], mybir.dt.float32)
            nc.vector.memset(eps_t, EPS)

            for t in range(NT):
                res = res_pool.tile([P, OUT_H2, 2, OUT_W], mybir.dt.float32)
                for oh in range(OUT_H2):
                    last = (t == NT - 1) and (oh == OUT_H2 - 1)
                    slab = slabs.tile([P, SLAB], mybir.dt.float32)
                    if not last:
                        nc.sync.dma_start(
                            out=slab[:],
                            in_=x[t * P:(t + 1) * P,
                                  oh * SLAB:(oh + 1) * SLAB],
                            single_packet=True,
                        )
                        # [p, (r w)] -> [p, ow, r, w] (ow=8, r=80, w=1 steps)
                        slab_v = slab.rearrange("p (r ow w) -> p ow r w",
                                                ow=OUT_W, w=WW)
                        stats = small.tile([P, OUT_W, 2, 6],
                                           mybir.dt.float32)
                        for ow in range(OUT_W):
                            for k in range(2):
                                bn_stats_raw(
                                    slab_v[:, ow, :, k * 4:(k + 1) * 4],
                                    stats[:, ow, k, :],
                                )
                            # (mean, var) straight into interleaved layout
                            nc.vector.bn_aggr(out=res[:, oh, :, ow],
                                              in_=stats[:, ow, :, :])
                    else:
                        # Final slab: split windows across DVE (bn_stats)
                        # and ACT (Square/Copy + accum) so both engines
                        # share the drain tail.
                        n_dve = 7
                        nc.sync.dma_start(
                            out=slab[:],
                            in_=x[t * P:(t + 1) * P,
                                  oh * SLAB:(oh + 1) * SLAB],
                            single_packet=True,
                        )
                        slab_v = slab.rearrange("p (r ow w) -> p ow r w",
                                                ow=OUT_W, w=WW)
                        stats = small.tile([P, OUT_W, 2, 6],
                                           mybir.dt.float32)
                        for ow in range(n_dve):
                            for k in range(2):
                                bn_stats_raw(
                                    slab_v[:, ow, :, k * 4:(k + 1) * 4],
                                    stats[:, ow, k, :],
                                )
                            nc.vector.bn_aggr(out=res[:, oh, :, ow],
                                              in_=stats[:, ow, :, :])
                        sums = small.tile([P, OUT_W], mybir.dt.float32)
                        sqs = small.tile([P, OUT_W], mybir.dt.float32)
                        for ow in range(n_dve, OUT_W):
                            sq_scr = scratch_pool.tile([P, WH, WW],
                                                       mybir.dt.float32)
                            nc.scalar.activation(
                                out=sq_scr[:],
                                in_=slab_v[:, ow],
                                func=mybir.ActivationFunctionType.Square,
                                accum_out=sqs[:, ow:ow + 1],
                            )
                            cp_scr = scratch_pool.tile([P, WH, WW],
                                                       mybir.dt.float32)
                            nc.scalar.activation(
                                out=cp_scr[:],
                                in_=slab_v[:, ow],
                                func=mybir.ActivationFunctionType.Copy,
                                accum_out=sums[:, ow:ow + 1],
                            )
                        # mean = sums/N ; var = sqs/N - mean^2
                        nc.vector.tensor_scalar_mul(
                            res[:, oh, 0, n_dve:], sums[:, n_dve:], INV_N)
                        m2 = small.tile([P, OUT_W], mybir.dt.float32)
                        nc.vector.tensor_mul(m2[:, n_dve:],
                                             res[:, oh, 0, n_dve:],
                                             res[:, oh, 0, n_dve:])
                        nc.vector.scalar_tensor_tensor(
                            out=res[:, oh, 1, n_dve:],
                            in0=sqs[:, n_dve:],
                            scalar=INV_N,
                            in1=m2[:, n_dve:],
                            op0=mybir.AluOpType.mult,
                            op1=mybir.AluOpType.subtract,
                        )
                    # std = sqrt(var + eps), in place over the var row
                    nc.scalar.activation(
                        out=res[:, oh, 1, :],
                        in_=res[:, oh, 1, :],
                        func=mybir.ActivationFunctionType.Sqrt,
                        bias=eps_t[:],
                        scale=1.0,
                    )
                nc.sync.dma_start(out=out[t * P:(t + 1) * P, :], in_=res[:])
    nc.compile()
    return nc


def _ensure_ntff_shim():
    """bass_utils imports antenv.axon_hooks when tracing is requested
    (trace=True or BASS_TRACE=1); some images lack that module. Provide a
    functional shim backed by trn_boot's ctypes NTFF hook when possible,
    else a no-op that degrades tracing gracefully."""
    import sys
    import types
    try:
        import antenv.axon_hooks  # noqa: F401
        return
    except ImportError:
        pass
    try:
        import antenv
    except ImportError:
        return
    mod = types.ModuleType("antenv.axon_hooks")
    mod._hook = None
    mod.set_axon_ntff_profile_hook = lambda h: setattr(mod, "_hook", h)
    mod.get_axon_ntff_profile_hook = lambda: mod._hook
    try:
        from trn_agent_boot.trn_boot import _ntff_profile_via_ctypes
        mod.set_axon_ntff_profile_hook(
            _ntff_profile_via_ctypes("/opt/axon/libaxon_pjrt.so"))
    except Exception:
        pass
    sys.modules["antenv.axon_hooks"] = mod
    antenv.axon_hooks = mod


def kernel(x: np.ndarray) -> np.ndarray:
    global LAST_RESULTS
    _ensure_ntff_shim()
    from concourse.bass_utils import run_bass_kernel_spmd

    if "nc" not in _CACHE:
        _CACHE["nc"] = _build()
    nc = _CACHE["nc"]

    x = np.ascontiguousarray(np.asarray(x, dtype=np.float32))
    in_maps = [
        {"x": x[i * B_SH:(i + 1) * B_SH].reshape(BC, HW)}
        for i in range(N_CORES)
    ]
    trace = bool(int(os.environ.get("KERNEL_TRACE", "0")))
    res = run_bass_kernel_spmd(nc, in_maps, core_ids=list(range(N_CORES)),
                               trace=trace)
    LAST_RESULTS = res
    out = np.concatenate(
        [res.results[i]["out"].reshape(B_SH, C, 2 * OUT_H2, OUT_W)
         for i in range(N_CORES)],
        axis=0,
    )
    return out


# revision 24
# speedup vs baseline: 1.0072x; 1.0072x over previous
"""AdaptiveStdPool2d kernel for Trainium2 (8 NeuronCores, data-parallel).

Input  x: [32, 64, 512, 80] f32
Output:   [32, 64, 8, 10] f32  (mean/std interleaved along height)

Math: per (b, c), split H=512 into 4 windows of 128 and W=80 into 10
windows of 8; out[b,c,2*oh,ow] = mean of 128x8 window, out[b,c,2*oh+1,ow]
= sqrt(biased_var + 1e-14).

Strategy (per core, shard = 4 batches -> 256 (b,c) rows):
- partition dim = (b,c) row (2 tiles of 128), free dim = flattened (h,w).
- per (bc-tile, oh): one big contiguous DMA slab [128, 10240] (40 KB/row).
- stats: per window, 2x BNStats (512-elem halves; HW flattens the
  multi-dim AP into one group) + BNStatsAggregate writing (mean, var)
  directly into the interleaved output layout. Single DVE pass gives
  both stats; ACT only does the final sqrt(var + eps) in place.
- last slab splits windows across DVE (bn_stats) and ACT (Square/Copy +
  accum_out) so the drain tail is shared by both engines.
"""

import os
import numpy as np

B, C, H, W = 32, 64, 512, 80
OUT_H2, OUT_W = 4, 10
WH, WW = H // OUT_H2, W // OUT_W  # 128, 8
EPS = 1e-14
NWIN = WH * WW                   # 1024 elements per window

N_CORES = 8
B_SH = B // N_CORES          # 4 batches per core
BC = B_SH * C                # 256 rows per core
HW = H * W                   # 40960
SLAB = WH * W                # 10240 elements per (oh) slab
OUT_FREE = 2 * OUT_H2 * OUT_W  # 80 output elements per (b,c)

_CACHE = {}
LAST_RESULTS = None


def _build():
    import concourse.bacc as bacc
    import concourse.tile as tile
    from concourse import mybir

    nc = bacc.Bacc("TRN2", target_bir_lowering=False, debug=False)

    x = nc.dram_tensor("x", [BC, HW], mybir.dt.float32, kind="ExternalInput")
    out = nc.dram_tensor("out", [BC, OUT_FREE], mybir.dt.float32,
                         kind="ExternalOutput")

    P = 128
    NT = BC // P  # 2 bc-tiles
    INV_N = 1.0 / NWIN

    def bn_stats_raw(in_ap, out_ap):
        # Raw InstBNStats: HW computes one 6-stat group over the whole
        # (multi-dim) input AP; bass's wrapper would reject this shape.
        nc.vector.add_instruction(
            mybir.InstBNStats(
                name=nc.get_next_instruction_name(),
                ins=[nc.vector.lower_ap(in_ap)],
                outs=[nc.vector.lower_ap(out_ap)],
            )
        )

    with tile.TileContext(nc) as tc:
        with (
            tc.tile_pool(name="slabs", bufs=4) as slabs,
            tc.tile_pool(name="scratch", bufs=2) as scratch_pool,
            tc.tile_pool(name="small", bufs=4) as small,
            tc.tile_pool(name="res", bufs=2) as res_pool,
            tc.tile_pool(name="singles", bufs=1) as singles,
        ):
            eps_t = singles.tile([P, 1], mybir.dt.float32)
            nc.vector.memset(eps_t, EPS)

            for t in range(NT):
                res = res_pool.tile([P, OUT_H2, 2, OUT_W], mybir.dt.float32)
                for oh in range(OUT_H2):
                    last = (t == NT - 1) and (oh == OUT_H2 - 1)
                    slab = slabs.tile([P, SLAB], mybir.dt.float32)
                    if not last:
                        nc.sync.dma_start(
                            out=slab[:],
                            in_=x[t * P:(t + 1) * P,
                                  oh * SLAB:(oh + 1) * SLAB],
                        )
                        # [p, (r w)] -> [p, ow, r, w] (ow=8, r=80, w=1 steps)
                        slab_v = slab.rearrange("p (r ow w) -> p ow r w",
                                                ow=OUT_W, w=WW)
                        stats = small.tile([P, OUT_W, 2, 6],
                                           mybir.dt.float32)
                        for ow in range(OUT_W):
                            for k in range(2):
                                bn_stats_raw(
                                    slab_v[:, ow, :, k * 4:(k + 1) * 4],
                                    stats[:, ow, k, :],
                                )
                            # (mean, var) straight into interleaved layout
                            nc.vector.bn_aggr(out=res[:, oh, :, ow],
                                              in_=stats[:, ow, :, :])
                    else:
                        # Final slab: split windows across DVE (bn_stats)
                        # and ACT (Square/Copy + accum) so both engines
                        # share the drain tail.
                        n_dve = 7
                        nc.sync.dma_start(
                            out=slab[:],
                            in_=x[t * P:(t + 1) * P,
                                  oh * SLAB:(oh + 1) * SLAB],
                        )
                        slab_v = slab.rearrange("p (r ow w) -> p ow r w",
                                                ow=OUT_W, w=WW)
                        stats = small.tile([P, OUT_W, 2, 6],
                                           mybir.dt.float32)
                        for ow in range(n_dve):
                            for k in range(2):
                                bn_stats_raw(
                                    slab_v[:, ow, :, k * 4:(k + 1) * 4],
                                    stats[:, ow, k, :],
                                )
                            nc.vector.bn_aggr(out=res[:, oh, :, ow],
                                              in_=stats[:, ow, :, :])
                        sums = small.tile([P, OUT_W], mybir.dt.float32)
                        sqs = small.tile([P, OUT_W], mybir.dt.float32)
                        for ow in range(n_dve, OUT_W):
                            sq_scr = scratch_pool.tile([P, WH, WW],
                                                       mybir.dt.float32)
                            nc.scalar.activation(
                                out=sq_scr[:],
                                in_=slab_v[:, ow],
                                func=mybir.ActivationFunctionType.Square,
                                accum_out=sqs[:, ow:ow + 1],
                            )
                            cp_scr = scratch_pool.tile([P, WH, WW],
                                                       mybir.dt.float32)
                            nc.scalar.activation(
                                out=cp_scr[:],
                                in_=slab_v[:, ow],
                                func=mybir.ActivationFunctionType.Copy,
                                accum_out=sums[:, ow:ow + 1],
                            )
                        # mean = sums/N ; var = sqs/N - mean^2
                        nc.vector.tensor_scalar_mul(
                            res[:, oh, 0, n_dve:], sums[:, n_dve:], INV_N)
                        m2 = small.tile([P, OUT_W], mybir.dt.float32)
                        nc.vector.tensor_mul(m2[:, n_dve:],
                                             res[:, oh, 0, n_dve:],
                                             res[:, oh, 0, n_dve:])
                        nc.vector.scalar_tensor_tensor(
                            out=res[:, oh, 1, n_dve:],
                            in0=sqs[:, n_dve:],
                            scalar=INV_N,
                            in1=m2[:, n_dve:],
                            op0=mybir.AluOpType.mult,
                            op1=mybir.AluOpType.subtract,
                        )
                    # std = sqrt(var + eps), in place over the var row
                    nc.scalar.activation(
                        out=res[:, oh, 1, :],
                        in_=res[:, oh, 1, :],
                        func=mybir.ActivationFunctionType.Sqrt,
                        bias=eps_t[:],
                        scale=1.0,
                    )
                nc.sync.dma_start(out=out[t * P:(t + 1) * P, :], in_=res[:])
    nc.compile()
    return nc


def _ensure_ntff_shim():
    """bass_utils imports antenv.axon_hooks when tracing is requested
    (trace=True or BASS_TRACE=1); some images lack that module. Provide a
    functional shim backed by trn_boot's ctypes NTFF hook when possible,
    else a no-op that degrades tracing gracefully."""
    import sys
    import types
    try:
        import antenv.axon_hooks  # noqa: F401
        return
    except ImportError:
        pass
    try:
        import antenv
    except ImportError:
        return
    mod = types.ModuleType("antenv.axon_hooks")
    mod._hook = None
    mod.set_axon_ntff_profile_hook = lambda h: setattr(mod, "_hook", h)
    mod.get_axon_ntff_profile_hook = lambda: mod._hook
    try:
        from trn_agent_boot.trn_boot import _ntff_profile_via_ctypes
        mod.set_axon_ntff_profile_hook(
            _ntff_profile_via_ctypes("/opt/axon/libaxon_pjrt.so"))
    except Exception:
        pass
    sys.modules["antenv.axon_hooks"] = mod
    antenv.axon_hooks = mod


def kernel(x: np.ndarray) -> np.ndarray:
    global LAST_RESULTS
    _ensure_ntff_shim()
    from concourse.bass_utils import run_bass_kernel_spmd

    if "nc" not in _CACHE:
        _CACHE["nc"] = _build()
    nc = _CACHE["nc"]

    x = np.ascontiguousarray(np.asarray(x, dtype=np.float32))
    in_maps = [
        {"x": x[i * B_SH:(i + 1) * B_SH].reshape(BC, HW)}
        for i in range(N_CORES)
    ]
    trace = bool(int(os.environ.get("KERNEL_TRACE", "0")))
    res = run_bass_kernel_spmd(nc, in_maps, core_ids=list(range(N_CORES)),
                               trace=trace)
    LAST_RESULTS = res
    out = np.concatenate(
        [res.results[i]["out"].reshape(B_SH, C, 2 * OUT_H2, OUT_W)
         for i in range(N_CORES)],
        axis=0,
    )
    return out
